# revision 40
# baseline (speedup 1.0000x reference)
"""Trainium2 Bass kernel for nn_DenseCondenser (TT contraction, 65536x4096 -> 65536x8).

The three (8,8,8) TT cores compose into a single effective matrix E (4096, 8)
(the whole map is linear in x), folded on host in float64. The device kernel
is then a memory-bound skinny matmul out = x @ E + bias, data-parallel over
the batch across 8 NeuronCores.

MODE "fp8e3" (default): x is cast on host to fp8 e3m4 (Trainium FP8_EXP3,
4 mantissa bits) with a power-of-2 scale folded into E; E stays fp16
(TensorE allows mixed input dtypes; both upcast to ~fp22 internally).
This halves HBM traffic vs fp16 (L2 rel err ~1.3e-2 vs the 2e-2 gate).
At 1 B/elem the PE streaming time (1 col/cycle, only 8 of 128 array
columns used) would exceed the DMA time, so the matmuls are packed 4x
with PE column tiling: col group g (tile_position=(0,32g)) processes
batch slice g of the chunk, writing psum partitions 32g..32g+8. A single
full-width (M=128) bias-broadcast matmul opens each PSUM bank (start=True
clears has_written for the WHOLE bank, so it must happen exactly once per
bank, before all 4 groups' accumulating matmuls). DMA cannot read PSUM,
so one full-width DVE copy evacuates psum->sbuf, then 4 stores (one per
col group's partition range) ride the Scalar HWDGE ring.

Device-side layout: x is staged per-core host-blocked as
xb (8 chunks, 128 partitions, 32 ktiles, 1024 batch) so the contraction
dim lands on SBUF partitions and every (chunk, partition) DMA payload is
one contiguous 32 KiB fp8 run. All chunk loads are hoisted up front on
the Sync HWDGE ring with a deep tile pool; the last chunk loads in halves
to shrink the end-of-stream completion gate.

Baseline history: fp32 363 us -> fp16 ~183-223 us -> fp8e3 (this).
"""

import numpy as np
import ml_dtypes

import concourse.bass as bass
import concourse.mybir as mybir
import concourse.tile as tile
from concourse import bacc
from concourse.bass import ts
from concourse.bass_utils import run_bass_kernel_spmd

# Problem shapes (hardcoded per harness contract)
BATCH = 65536
K = 4096  # input features = 8**4
C = 8  # output features
N_CORES = 8
B_CORE = BATCH // N_CORES  # 8192
NK = K // 128  # 32 k-tiles

# fp8e3 mode geometry: 1024-batch chunks, 4 PE col groups x 256-batch slices
CHUNK8 = 1024
NCHUNK8 = B_CORE // CHUNK8  # 8
NGRP = 4
NSLICE = CHUNK8 // NGRP  # 256
# filler matmuls appended after each chunk's real work: they keep the PE's
# HAM activity window busy through the DMA-bound stretches so the final
# (load-gated) rounds run at 2.4 GHz instead of re-throttled 1.2 GHz
FILL_ROUNDS = 0
MID_FILL = 12

# fp16 mode geometry (legacy fallback)
CHUNK16 = 512
NCHUNK16 = B_CORE // CHUNK16  # 16

# x quantization scale for fp8e3 (power of 2, folded into E). At s=2 the
# e3m4 normal range [0.25, 15.5] covers [0.125, 7.75] sigma: no clipping
# in practice (max|x| ~ 5.6), subnormal floor negligible.
SCALE = 2.0

MODE = "fp8e3"

_program_cache = {}


def _build_program_fp8(mode: str) -> bass.Bass:
    f32 = mybir.dt.float32
    f16 = mybir.dt.float16
    f8 = mybir.dt.float8e3
    nc = bacc.Bacc(None, name="dense_condenser")

    # xb[j, p, kt, b] = xq[j*CHUNK8 + b, kt*128 + p]: per (chunk, partition)
    # the (kt, b) payload is one contiguous 32 KiB fp8 run.
    xb = nc.dram_tensor("xb", (NCHUNK8, 128, NK, CHUNK8), f8, kind="ExternalInput")
    eb = nc.dram_tensor("eb", (128, NK, C), f16, kind="ExternalInput")
    # biasw[0, 32g+c] = bias[c] for g in 0..3, zeros elsewhere: the
    # stationary operand of the bank-opening broadcast matmul.
    biasw = nc.dram_tensor("biasw", (1, 128), f16, kind="ExternalInput")
    ones = nc.dram_tensor("ones", (1, NSLICE), f16, kind="ExternalInput")
    # full-width output staging: partition 32g+c, chunk ch, col b holds
    # out[ch*CHUNK8 + g*NSLICE + b, c]; partitions outside the 4 live
    # 8-row ranges carry bias junk the host discards. Storing all 128
    # partitions keeps it to ONE ~600ns HWDGE trigger per chunk instead
    # of 4 (the extra bytes are trivial: 64 KiB/chunk at 358 GB/s).
    outF = nc.dram_tensor("outF", (128, NCHUNK8, NSLICE), f16, kind="ExternalOutput")

    with tile.TileContext(nc) as tc:
        with (
            tc.tile_pool(name="consts", bufs=1) as consts,
            tc.tile_pool(name="xp", bufs=5) as xp,
            tc.tile_pool(name="op", bufs=2) as op,
            tc.tile_pool(name="pp", bufs=2, space=bass.MemorySpace.PSUM) as pp,
            tc.tile_pool(name="pw", bufs=1, space=bass.MemorySpace.PSUM) as pw,
        ):
            e_tile = consts.tile([128, NK, C], f16)
            biasw_tile = consts.tile([1, 128], f16)
            ones_tile = consts.tile([1, NSLICE], f16)

            # x loads stream on the Sync HWDGE ring; consts ride the Scalar
            # ring so chunk 0's load is the very first thing the Sync ring
            # processes.
            x_tiles = []
            for j in range(NCHUNK8):
                x_tile = xp.tile([128, NK, CHUNK8], f8)
                x_tiles.append(x_tile)
                if j == 0:
                    nc.sync.dma_start(out=x_tile[:], in_=xb[j])
                    nc.scalar.dma_start(out=e_tile[:], in_=eb[:])
                    nc.scalar.dma_start(out=biasw_tile[:], in_=biasw[:])
                    nc.scalar.dma_start(out=ones_tile[:], in_=ones[:])
                elif j < NCHUNK8 - 1:
                    # whole-chunk loads: one 32 KiB contiguous run per
                    # partition is the DMA sweet spot (16 KiB and 64 KiB
                    # runs both measured slower).
                    nc.sync.dma_start(out=x_tile[:], in_=xb[j])
                else:
                    # final chunk in 16-ktile halves (16 KiB/partition runs;
                    # 8-ktile gates measured slower: their 8 KiB runs hit the
                    # degenerate single-DMA-engine path).
                    for lo, hi in ((0, 16), (16, 32)):
                        nc.sync.dma_start(
                            out=x_tile[:, lo:hi], in_=xb[j, :, lo:hi]
                        )

            warm_tile = pw.tile([128, NSLICE], f32)

            def filler_rounds(n, x_tile):
                # Redundant matmuls into a scratch PSUM bank. No consumers,
                # no waits: the PE runs them during what would otherwise be
                # DMA-bound idle, keeping the HAM activity window busy so
                # gated bursts run at 2.4 GHz instead of re-throttled 1.2.
                for _ in range(n):
                    nc.tensor.matmul(
                        warm_tile[:C, :],
                        e_tile[:, 0, :],
                        x_tile[:, 0, ts(0, NSLICE)],
                        start=True,
                        stop=True,
                        skip_group_check=True,
                        tile_position=(0, 0),
                    )

            for ch in range(NCHUNK8):
                x_tile = x_tiles[ch]
                psum_tile = pp.tile([128, NSLICE], f32)
                # Bank-wide opener: out[32g+c, b] = bias[c], has_written set
                # for every element of the bank so the 4 interleaved col
                # groups below can all accumulate with start=False.
                nc.tensor.matmul(
                    psum_tile[:],
                    biasw_tile[:],
                    ones_tile[:],
                    start=True,
                    stop=False,
                    skip_group_check=True,
                )
                # 4 col groups run concurrently (distinct 32-col array
                # strips + own XBUS streams): group g contracts k-tile kt
                # for batch slice g. kt-outer / g-inner issue order keeps
                # consecutive PE instructions on distinct groups.
                for kt in range(NK):
                    last = kt == NK - 1
                    for g in range(NGRP):
                        nc.tensor.matmul(
                            psum_tile[32 * g : 32 * g + C, :],
                            e_tile[:, kt, :],
                            x_tile[:, kt, ts(g, NSLICE)],
                            start=False,
                            stop=last,
                            skip_group_check=True,
                            tile_position=(0, 32 * g),
                        )
                    if MID_FILL and ch == NCHUNK8 - 1 and kt == 15:
                        # bridge the gate-A -> gate-B idle (warm-state HAM
                        # re-throttles after ~1.7 us of PE idle)
                        filler_rounds(MID_FILL, x_tile)

                # One full-width evacuation on ScalarE (psum partitions 8..31
                # etc. hold bias junk; the stores below pick the 4 live
                # ranges). ScalarE is also the store-trigger engine, so its
                # stores follow the evac in program order with no cross-
                # engine semaphore hop; output downcast to fp16 (rel err
                # ~5e-4, negligible vs the fp8 x quantization) halves the
                # store bytes.
                out_tile = op.tile([128, NSLICE], f16, tag="out")
                nc.vector.tensor_scalar_add(out_tile[:], psum_tile[:], 0.0)
                # mid-run stores hide under the load stream on the Scalar
                # ring; the final one rides the (by then idle) Sync ring.
                nc.scalar.dma_start(out=outF[:, ch, :], in_=out_tile[:])
                if FILL_ROUNDS and ch < NCHUNK8 - 1:
                    filler_rounds(FILL_ROUNDS, x_tile)

    nc.compile()
    return nc


def _build_program_fp16(mode: str) -> bass.Bass:
    """Legacy fp16 program (see git history for rationale); kept as fallback."""
    f32 = mybir.dt.float32
    mmdt = mybir.dt.float16
    nc = bacc.Bacc(None, name="dense_condenser")

    xb = nc.dram_tensor("xb", (NCHUNK16, 128, NK, CHUNK16), mmdt, kind="ExternalInput")
    eb = nc.dram_tensor("eb", (128, NK, C), mmdt, kind="ExternalInput")
    bias = nc.dram_tensor("bias", (C, 1), f32, kind="ExternalInput")
    outT = nc.dram_tensor("outT", (C, B_CORE), f32, kind="ExternalOutput")

    with tile.TileContext(nc) as tc:
        with (
            tc.tile_pool(name="consts", bufs=1) as consts,
            tc.tile_pool(name="xp", bufs=5) as xp,
            tc.tile_pool(name="op", bufs=2) as op,
            tc.tile_pool(name="pp", bufs=2, space=bass.MemorySpace.PSUM) as pp,
        ):
            e_tile = consts.tile([128, NK, C], mmdt)
            bias_tile = consts.tile([C, 1], f32)

            x_tiles = []
            for j in range(NCHUNK16):
                x_tile = xp.tile([128, NK, CHUNK16], mmdt)
                x_tiles.append(x_tile)
                if j == 0:
                    nc.sync.dma_start(out=x_tile[:], in_=xb[j])
                    nc.scalar.dma_start(out=bias_tile[:], in_=bias[:])
                    nc.scalar.dma_start(out=e_tile[:], in_=eb[:])
                elif j < NCHUNK16 - 2:
                    nc.sync.dma_start(out=x_tile[:], in_=xb[j])
                else:
                    nc.sync.dma_start(out=x_tile[:, : NK // 2], in_=xb[j, :, : NK // 2])
                    nc.sync.dma_start(out=x_tile[:, NK // 2 :], in_=xb[j, :, NK // 2 :])

            GROUP = 4
            out_tile = None
            for j in range(NCHUNK16):
                x_tile = x_tiles[j]
                psum_tile = pp.tile([C, CHUNK16], f32)
                for kt in range(NK):
                    nc.tensor.matmul(
                        psum_tile[:],
                        e_tile[:, kt, :],
                        x_tile[:, kt, :],
                        start=(kt == 0),
                        stop=(kt == NK - 1),
                    )

                if j % GROUP == 0:
                    out_tile = op.tile([C, GROUP * CHUNK16], f32, tag="out")
                nc.vector.tensor_scalar_add(
                    out_tile[:, ts(j % GROUP, CHUNK16)], psum_tile[:], bias_tile[:]
                )
                if j % GROUP == GROUP - 1:
                    nc.scalar.dma_start(
                        out=outT[:, ts(j // GROUP, GROUP * CHUNK16)], in_=out_tile[:]
                    )

    nc.compile()
    return nc


def _fold_E(node_0, node_1, node_2) -> np.ndarray:
    # E[(i,j,k,l), c3] = sum_{c1,c2} node_0[l,k,c1] node_1[c1,j,c2] node_2[c2,i,c3]
    E = np.einsum(
        "lkc,cjd,die->ijkle",
        node_0.astype(np.float64),
        node_1.astype(np.float64),
        node_2.astype(np.float64),
    )
    return E.reshape(K, C).astype(np.float32)


def kernel(x, node_0, node_1, node_2, bias, _trace=False, _trace_cores=None):
    x = np.asarray(x, dtype=np.float32)
    E = _fold_E(np.asarray(node_0), np.asarray(node_1), np.asarray(node_2))
    bias_np = np.asarray(bias, dtype=np.float32)

    if MODE not in _program_cache:
        _program_cache[MODE] = (
            _build_program_fp8(MODE) if MODE == "fp8e3" else _build_program_fp16(MODE)
        )
    nc = _program_cache[MODE]

    in_maps = []
    if MODE == "fp8e3":
        # blocked E with the x-scale folded out: eb[p, kt, c] = E[kt*128+p, c]/SCALE
        ebq = np.ascontiguousarray(
            (E / SCALE).reshape(NK, 128, C).transpose(1, 0, 2)
        ).astype(np.float16)
        biasw = np.zeros((1, 128), dtype=np.float16)
        for g in range(NGRP):
            biasw[0, 32 * g : 32 * g + C] = bias_np.astype(np.float16)
        ones = np.ones((1, NSLICE), dtype=np.float16)

        xq = np.clip(x * SCALE, -15.5, 15.5).astype(ml_dtypes.float8_e3m4)
        for m in range(N_CORES):
            x_m = xq[m * B_CORE : (m + 1) * B_CORE, :]
            # xb[j, p, kt, b] = x_m[j*CHUNK8 + b, kt*128 + p]
            xb_m = np.ascontiguousarray(
                x_m.reshape(NCHUNK8, CHUNK8, NK, 128).transpose(0, 3, 2, 1)
            )
            in_maps.append({"xb": xb_m, "eb": ebq, "biasw": biasw, "ones": ones})
    else:
        eb = np.ascontiguousarray(E.reshape(NK, 128, C).transpose(1, 0, 2)).astype(
            np.float16
        )
        bias_col = bias_np.reshape(C, 1)
        for m in range(N_CORES):
            x_m = x[m * B_CORE : (m + 1) * B_CORE, :]
            xb_m = x_m.reshape(NCHUNK16, CHUNK16, NK, 128).transpose(0, 3, 2, 1)
            xb_m = xb_m.astype(np.float16)
            in_maps.append({"xb": xb_m, "eb": eb, "bias": bias_col})

    res = run_bass_kernel_spmd(
        nc,
        in_maps,
        core_ids=list(range(N_CORES)),
        trace=_trace,
        trace_cores=_trace_cores,
    )
    results = res.results

    out = np.empty((BATCH, C), dtype=np.float32)
    for m in range(N_CORES):
        if MODE == "fp8e3":
            # outF[32g+c, ch, b] -> out[ch*CHUNK8 + g*NSLICE + b, c]
            arr = results[m]["outF"].reshape(NGRP, 32, NCHUNK8, NSLICE)[:, :C]
            out[m * B_CORE : (m + 1) * B_CORE, :] = (
                arr.transpose(2, 0, 3, 1).reshape(B_CORE, C).astype(np.float32)
            )
        else:
            out[m * B_CORE : (m + 1) * B_CORE, :] = results[m]["outT"].T.astype(
                np.float32
            )

    if _trace:
        return out, res
    return out


# revision 43
# speedup vs baseline: 1.0660x; 1.0660x over previous
"""Trainium2 Bass kernel for nn_DenseCondenser (TT contraction, 65536x4096 -> 65536x8).

The three (8,8,8) TT cores compose into a single effective matrix E (4096, 8)
(the whole map is linear in x), folded on host in float64. The device kernel
is then a memory-bound skinny matmul out = x @ E + bias, data-parallel over
the batch across 8 NeuronCores.

MODE "fp8e3" (default): x is cast on host to fp8 e3m4 (Trainium FP8_EXP3,
4 mantissa bits) with a power-of-2 scale folded into E; E stays fp16
(TensorE allows mixed input dtypes; both upcast to ~fp22 internally).
This halves HBM traffic vs fp16 (L2 rel err 1.332e-2 vs the 2e-2 gate;
fp16 was 2.9e-4). At 1 B/elem the PE streaming time (1 col/cycle, only
8 of 128 array columns used) would exceed the DMA time, so the matmuls
are packed 4x with PE column tiling: col group g (tile_position=(0,32g))
processes batch slice g of the chunk, writing psum partitions 32g..32g+8
(no cross-group combine needed - the groups are just different batch
rows). A single full-width (M=128) bias-broadcast matmul opens each PSUM
bank (start=True clears has_written for the WHOLE bank, so it must happen
exactly once per bank, before all 4 groups' accumulating matmuls). DMA
cannot read PSUM, so one full-width DVE copy (fp32 psum -> fp16 sbuf)
evacuates each chunk, then ONE 128-partition store per chunk writes the
staging tensor outF (junk partitions included - trivial bytes, and 1
HWDGE trigger at ~0.6 us each beats 4); the host picks the live rows.

Device-side layout: x is staged per-core host-blocked as
xb (8 chunks, 128 partitions, 32 ktiles, 1024 batch) so the contraction
dim lands on SBUF partitions and every (chunk, partition) DMA payload is
one contiguous 32 KiB fp8 run (16 KiB and 64 KiB runs both measured
slower; 8 KiB runs hit a degenerate single-DMA-engine path). All chunk
loads are hoisted up front on the Sync HWDGE ring (bufs=5; bufs=6
regressed); mid-run stores ride the Scalar ring, the final store the
then-idle Sync ring. The last chunk loads in 16-ktile halves so only 16
rounds remain after the final byte, and MID_FILL scratch matmuls keep
the PE's HAM clock gate warm across the inter-gate idle.

Measured (8-way SPMD, profiled core): 107.2 us best / ~14.8 us overhead
above the DMA floor; per-core sustained load rate swings 92-109 us for
the same 32 MiB with chip load/thermals, so absolute exec varies
run-to-run. History: fp32 363 -> fp16 210-223 -> fp8e3 107-118.
NOTE: per-chunk warm-keeper fillers, 8-ktile gates, scalar-engine evac,
and store-per-group variants each measured SLOWER - see git history.
"""

import numpy as np
import ml_dtypes

import concourse.bass as bass
import concourse.mybir as mybir
import concourse.tile as tile
from concourse import bacc
from concourse.bass import ts
from concourse.bass_utils import run_bass_kernel_spmd

# Problem shapes (hardcoded per harness contract)
BATCH = 65536
K = 4096  # input features = 8**4
C = 8  # output features
N_CORES = 8
B_CORE = BATCH // N_CORES  # 8192
NK = K // 128  # 32 k-tiles

# fp8e3 mode geometry: 1024-batch chunks, 4 PE col groups x 256-batch slices
CHUNK8 = 1024
NCHUNK8 = B_CORE // CHUNK8  # 8
NGRP = 4
NSLICE = CHUNK8 // NGRP  # 256
# filler matmuls bridging the last chunk's gate-A -> gate-B PE idle: the
# HAM clock gate re-throttles the PE to 1.2 GHz after ~1.7 us of warm-state
# idle, and the final 16 load-gated rounds are on the critical path. 12
# rounds (~1.4 us) keep the idle under the window. Fillers anywhere else
# (per-chunk) measurably SLOW THE DMA STREAM (~+5 us) - do not add them.
MID_FILL = 12

# fp16 mode geometry (legacy fallback)
CHUNK16 = 512
NCHUNK16 = B_CORE // CHUNK16  # 16

# x quantization scale for fp8e3 (power of 2, folded into E). At s=2 the
# e3m4 normal range [0.25, 15.5] covers [0.125, 7.75] sigma: no clipping
# in practice (max|x| ~ 5.6), subnormal floor negligible.
SCALE = 2.0

MODE = "fp8e3"

_program_cache = {}


def _build_program_fp8(mode: str) -> bass.Bass:
    f32 = mybir.dt.float32
    f16 = mybir.dt.float16
    f8 = mybir.dt.float8e3
    nc = bacc.Bacc(None, name="dense_condenser")

    # xb[j, p, kt, b] = xq[j*CHUNK8 + b, kt*128 + p]: per (chunk, partition)
    # the (kt, b) payload is one contiguous 32 KiB fp8 run.
    xb = nc.dram_tensor("xb", (NCHUNK8, 128, NK, CHUNK8), f8, kind="ExternalInput")
    eb = nc.dram_tensor("eb", (128, NK, C), f16, kind="ExternalInput")
    # biasw[0, 32g+c] = bias[c] for g in 0..3, zeros elsewhere: the
    # stationary operand of the bank-opening broadcast matmul.
    biasw = nc.dram_tensor("biasw", (1, 128), f16, kind="ExternalInput")
    ones = nc.dram_tensor("ones", (1, NSLICE), f16, kind="ExternalInput")
    # full-width output staging: partition 32g+c, chunk ch, col b holds
    # out[ch*CHUNK8 + g*NSLICE + b, c]; partitions outside the 4 live
    # 8-row ranges carry bias junk the host discards. Storing all 128
    # partitions keeps it to ONE ~600ns HWDGE trigger per chunk instead
    # of 4 (the extra bytes are trivial: 64 KiB/chunk at 358 GB/s).
    outF = nc.dram_tensor("outF", (128, NCHUNK8, NSLICE), f16, kind="ExternalOutput")

    with tile.TileContext(nc) as tc:
        with (
            tc.tile_pool(name="consts", bufs=1) as consts,
            tc.tile_pool(name="xp", bufs=5) as xp,
            tc.tile_pool(name="op", bufs=2) as op,
            tc.tile_pool(name="pp", bufs=2, space=bass.MemorySpace.PSUM) as pp,
            tc.tile_pool(name="pw", bufs=1, space=bass.MemorySpace.PSUM) as pw,
        ):
            e_tile = consts.tile([128, NK, C], f16)
            biasw_tile = consts.tile([1, 128], f16)
            ones_tile = consts.tile([1, NSLICE], f16)

            # x loads stream on the Sync HWDGE ring; consts ride the Scalar
            # ring so chunk 0's load is the very first thing the Sync ring
            # processes.
            x_tiles = []
            for j in range(NCHUNK8):
                x_tile = xp.tile([128, NK, CHUNK8], f8)
                x_tiles.append(x_tile)
                if j == 0:
                    nc.sync.dma_start(out=x_tile[:], in_=xb[j])
                    nc.scalar.dma_start(out=e_tile[:], in_=eb[:])
                    nc.scalar.dma_start(out=biasw_tile[:], in_=biasw[:])
                    nc.scalar.dma_start(out=ones_tile[:], in_=ones[:])
                elif j < NCHUNK8 - 1:
                    # whole-chunk loads: one 32 KiB contiguous run per
                    # partition is the DMA sweet spot (16 KiB and 64 KiB
                    # runs both measured slower).
                    nc.sync.dma_start(out=x_tile[:], in_=xb[j])
                else:
                    # final chunk in 16-ktile halves (16 KiB/partition runs;
                    # 8-ktile gates measured slower: their 8 KiB runs hit the
                    # degenerate single-DMA-engine path).
                    for lo, hi in ((0, 16), (16, 32)):
                        nc.sync.dma_start(
                            out=x_tile[:, lo:hi], in_=xb[j, :, lo:hi]
                        )

            warm_tile = pw.tile([128, NSLICE], f32)

            def filler_rounds(n, x_tile):
                # Redundant matmuls into a scratch PSUM bank. No consumers,
                # no waits: the PE runs them during what would otherwise be
                # DMA-bound idle, keeping the HAM activity window busy so
                # gated bursts run at 2.4 GHz instead of re-throttled 1.2.
                for _ in range(n):
                    nc.tensor.matmul(
                        warm_tile[:C, :],
                        e_tile[:, 0, :],
                        x_tile[:, 0, ts(0, NSLICE)],
                        start=True,
                        stop=True,
                        skip_group_check=True,
                        tile_position=(0, 0),
                    )

            for ch in range(NCHUNK8):
                x_tile = x_tiles[ch]
                psum_tile = pp.tile([128, NSLICE], f32)
                # Bank-wide opener: out[32g+c, b] = bias[c], has_written set
                # for every element of the bank so the 4 interleaved col
                # groups below can all accumulate with start=False.
                nc.tensor.matmul(
                    psum_tile[:],
                    biasw_tile[:],
                    ones_tile[:],
                    start=True,
                    stop=False,
                    skip_group_check=True,
                )
                # 4 col groups run concurrently (distinct 32-col array
                # strips + own XBUS streams): group g contracts k-tile kt
                # for batch slice g. kt-outer / g-inner issue order keeps
                # consecutive PE instructions on distinct groups.
                for kt in range(NK):
                    last = kt == NK - 1
                    for g in range(NGRP):
                        nc.tensor.matmul(
                            psum_tile[32 * g : 32 * g + C, :],
                            e_tile[:, kt, :],
                            x_tile[:, kt, ts(g, NSLICE)],
                            start=False,
                            stop=last,
                            skip_group_check=True,
                            tile_position=(0, 32 * g),
                        )
                    if ch == NCHUNK8 - 1 and kt == 15:
                        # bridge the gate-A -> gate-B idle (warm-state HAM
                        # re-throttles after ~1.7 us of PE idle)
                        filler_rounds(MID_FILL, x_tile)

                # One full-width DVE evacuation (psum partitions 8..31 etc.
                # hold bias junk; the host discards them). Downcast to fp16
                # (rel err ~5e-4, negligible vs the fp8 x quantization)
                # halves the store bytes. Evac on ScalarE measured slower
                # (its sequencer stall blocks the store triggers).
                out_tile = op.tile([128, NSLICE], f16, tag="out")
                nc.vector.tensor_scalar_add(out_tile[:], psum_tile[:], 0.0)
                # mid-run stores hide under the load stream on the Scalar
                # ring; the final one rides the (by then idle) Sync ring.
                eng = nc.sync if ch == NCHUNK8 - 1 else nc.scalar
                eng.dma_start(out=outF[:, ch, :], in_=out_tile[:])

    nc.compile()
    return nc


def _build_program_fp16(mode: str) -> bass.Bass:
    """Legacy fp16 program (see git history for rationale); kept as fallback."""
    f32 = mybir.dt.float32
    mmdt = mybir.dt.float16
    nc = bacc.Bacc(None, name="dense_condenser")

    xb = nc.dram_tensor("xb", (NCHUNK16, 128, NK, CHUNK16), mmdt, kind="ExternalInput")
    eb = nc.dram_tensor("eb", (128, NK, C), mmdt, kind="ExternalInput")
    bias = nc.dram_tensor("bias", (C, 1), f32, kind="ExternalInput")
    outT = nc.dram_tensor("outT", (C, B_CORE), f32, kind="ExternalOutput")

    with tile.TileContext(nc) as tc:
        with (
            tc.tile_pool(name="consts", bufs=1) as consts,
            tc.tile_pool(name="xp", bufs=5) as xp,
            tc.tile_pool(name="op", bufs=2) as op,
            tc.tile_pool(name="pp", bufs=2, space=bass.MemorySpace.PSUM) as pp,
        ):
            e_tile = consts.tile([128, NK, C], mmdt)
            bias_tile = consts.tile([C, 1], f32)

            x_tiles = []
            for j in range(NCHUNK16):
                x_tile = xp.tile([128, NK, CHUNK16], mmdt)
                x_tiles.append(x_tile)
                if j == 0:
                    nc.sync.dma_start(out=x_tile[:], in_=xb[j])
                    nc.scalar.dma_start(out=bias_tile[:], in_=bias[:])
                    nc.scalar.dma_start(out=e_tile[:], in_=eb[:])
                elif j < NCHUNK16 - 2:
                    nc.sync.dma_start(out=x_tile[:], in_=xb[j])
                else:
                    nc.sync.dma_start(out=x_tile[:, : NK // 2], in_=xb[j, :, : NK // 2])
                    nc.sync.dma_start(out=x_tile[:, NK // 2 :], in_=xb[j, :, NK // 2 :])

            GROUP = 4
            out_tile = None
            for j in range(NCHUNK16):
                x_tile = x_tiles[j]
                psum_tile = pp.tile([C, CHUNK16], f32)
                for kt in range(NK):
                    nc.tensor.matmul(
                        psum_tile[:],
                        e_tile[:, kt, :],
                        x_tile[:, kt, :],
                        start=(kt == 0),
                        stop=(kt == NK - 1),
                    )

                if j % GROUP == 0:
                    out_tile = op.tile([C, GROUP * CHUNK16], f32, tag="out")
                nc.vector.tensor_scalar_add(
                    out_tile[:, ts(j % GROUP, CHUNK16)], psum_tile[:], bias_tile[:]
                )
                if j % GROUP == GROUP - 1:
                    nc.scalar.dma_start(
                        out=outT[:, ts(j // GROUP, GROUP * CHUNK16)], in_=out_tile[:]
                    )

    nc.compile()
    return nc


def _fold_E(node_0, node_1, node_2) -> np.ndarray:
    # E[(i,j,k,l), c3] = sum_{c1,c2} node_0[l,k,c1] node_1[c1,j,c2] node_2[c2,i,c3]
    E = np.einsum(
        "lkc,cjd,die->ijkle",
        node_0.astype(np.float64),
        node_1.astype(np.float64),
        node_2.astype(np.float64),
    )
    return E.reshape(K, C).astype(np.float32)


def kernel(x, node_0, node_1, node_2, bias, _trace=False, _trace_cores=None):
    x = np.asarray(x, dtype=np.float32)
    E = _fold_E(np.asarray(node_0), np.asarray(node_1), np.asarray(node_2))
    bias_np = np.asarray(bias, dtype=np.float32)

    if MODE not in _program_cache:
        _program_cache[MODE] = (
            _build_program_fp8(MODE) if MODE == "fp8e3" else _build_program_fp16(MODE)
        )
    nc = _program_cache[MODE]

    in_maps = []
    if MODE == "fp8e3":
        # blocked E with the x-scale folded out: eb[p, kt, c] = E[kt*128+p, c]/SCALE
        ebq = np.ascontiguousarray(
            (E / SCALE).reshape(NK, 128, C).transpose(1, 0, 2)
        ).astype(np.float16)
        biasw = np.zeros((1, 128), dtype=np.float16)
        for g in range(NGRP):
            biasw[0, 32 * g : 32 * g + C] = bias_np.astype(np.float16)
        ones = np.ones((1, NSLICE), dtype=np.float16)

        xq = np.clip(x * SCALE, -15.5, 15.5).astype(ml_dtypes.float8_e3m4)
        for m in range(N_CORES):
            x_m = xq[m * B_CORE : (m + 1) * B_CORE, :]
            # xb[j, p, kt, b] = x_m[j*CHUNK8 + b, kt*128 + p]
            xb_m = np.ascontiguousarray(
                x_m.reshape(NCHUNK8, CHUNK8, NK, 128).transpose(0, 3, 2, 1)
            )
            in_maps.append({"xb": xb_m, "eb": ebq, "biasw": biasw, "ones": ones})
    else:
        eb = np.ascontiguousarray(E.reshape(NK, 128, C).transpose(1, 0, 2)).astype(
            np.float16
        )
        bias_col = bias_np.reshape(C, 1)
        for m in range(N_CORES):
            x_m = x[m * B_CORE : (m + 1) * B_CORE, :]
            xb_m = x_m.reshape(NCHUNK16, CHUNK16, NK, 128).transpose(0, 3, 2, 1)
            xb_m = xb_m.astype(np.float16)
            in_maps.append({"xb": xb_m, "eb": eb, "bias": bias_col})

    res = run_bass_kernel_spmd(
        nc,
        in_maps,
        core_ids=list(range(N_CORES)),
        trace=_trace,
        trace_cores=_trace_cores,
    )
    results = res.results

    out = np.empty((BATCH, C), dtype=np.float32)
    for m in range(N_CORES):
        if MODE == "fp8e3":
            # outF[32g+c, ch, b] -> out[ch*CHUNK8 + g*NSLICE + b, c]
            arr = results[m]["outF"].reshape(NGRP, 32, NCHUNK8, NSLICE)[:, :C]
            out[m * B_CORE : (m + 1) * B_CORE, :] = (
                arr.transpose(2, 0, 3, 1).reshape(B_CORE, C).astype(np.float32)
            )
        else:
            out[m * B_CORE : (m + 1) * B_CORE, :] = results[m]["outT"].T.astype(
                np.float32
            )

    if _trace:
        return out, res
    return out


# revision 46
# speedup vs baseline: 1.1193x; 1.0500x over previous
"""Trainium2 Bass kernel for nn_DenseCondenser (TT contraction, 65536x4096 -> 65536x8).

The three (8,8,8) TT cores compose into a single effective matrix E (4096, 8)
(the whole map is linear in x), folded on host in float64. The device kernel
is then a memory-bound skinny matmul out = x @ E + bias, data-parallel over
the batch across 8 NeuronCores.

MODE "fp8e3" (default): x is cast on host to fp8 e3m4 (Trainium FP8_EXP3,
4 mantissa bits) with a power-of-2 scale folded into E; E stays fp16
(TensorE allows mixed input dtypes; both upcast to ~fp22 internally).
This halves HBM traffic vs fp16 (L2 rel err 1.332e-2 vs the 2e-2 gate;
fp16 was 2.9e-4). At 1 B/elem the PE streaming time (1 col/cycle, only
8 of 128 array columns used) would exceed the DMA time, so the matmuls
are packed 4x with PE column tiling: col group g (tile_position=(0,32g))
processes batch slice g of the chunk, writing psum partitions 32g..32g+8
(no cross-group combine needed - the groups are just different batch
rows). A single full-width (M=128) bias-broadcast matmul opens each PSUM
bank (start=True clears has_written for the WHOLE bank, so it must happen
exactly once per bank, before all 4 groups' accumulating matmuls). DMA
cannot read PSUM, so one full-width DVE copy (fp32 psum -> fp16 sbuf)
evacuates each chunk, then ONE 128-partition store per chunk writes the
staging tensor outF (junk partitions included - trivial bytes, and 1
HWDGE trigger at ~0.6 us each beats 4); the host picks the live rows.

Device-side layout: x is staged per-core host-blocked as
xb (8 chunks, 128 partitions, 32 ktiles, 1024 batch) so the contraction
dim lands on SBUF partitions and every (chunk, partition) DMA payload is
one contiguous 32 KiB fp8 run (16 KiB and 64 KiB runs both measured
slower; 8 KiB runs hit a degenerate single-DMA-engine path). All chunk
loads are hoisted up front on the Sync HWDGE ring (bufs=5; bufs=6
regressed); mid-run stores ride the Scalar ring, the final store the
then-idle Sync ring. The last chunk loads in 16-ktile halves so only 16
rounds remain after the final byte, and MID_FILL scratch matmuls keep
the PE's HAM clock gate warm across the inter-gate idle.

Measured (8-way SPMD, profiled core): 107.2 us best / ~14.8 us overhead
above the DMA floor; per-core sustained load rate swings 92-109 us for
the same 32 MiB with chip load/thermals, so absolute exec varies
run-to-run. History: fp32 363 -> fp16 210-223 -> fp8e3 107-118.
NOTE: per-chunk warm-keeper fillers, 8-ktile gates, scalar-engine evac,
and store-per-group variants each measured SLOWER - see git history.
"""

import numpy as np
import ml_dtypes

import concourse.bass as bass
import concourse.mybir as mybir
import concourse.tile as tile
from concourse import bacc
from concourse.bass import ts
from concourse.bass_utils import run_bass_kernel_spmd

# Problem shapes (hardcoded per harness contract)
BATCH = 65536
K = 4096  # input features = 8**4
C = 8  # output features
N_CORES = 8
B_CORE = BATCH // N_CORES  # 8192
NK = K // 128  # 32 k-tiles

# fp8e3 mode geometry: 1024-batch chunks, 4 PE col groups x 256-batch slices
CHUNK8 = 1024
NCHUNK8 = B_CORE // CHUNK8  # 8
NGRP = 4
NSLICE = CHUNK8 // NGRP  # 256
# filler matmuls bridging the last chunk's gate-A -> gate-B PE idle: the
# HAM clock gate re-throttles the PE to 1.2 GHz after ~1.7 us of warm-state
# idle, and the final 16 load-gated rounds are on the critical path. 12
# rounds (~1.4 us) keep the idle under the window. Fillers anywhere else
# (per-chunk) measurably SLOW THE DMA STREAM (~+5 us) - do not add them.
MID_FILL = 12

# fp16 mode geometry (legacy fallback)
CHUNK16 = 512
NCHUNK16 = B_CORE // CHUNK16  # 16

# x quantization scale for fp8e3 (power of 2, folded into E). At s=2 the
# e3m4 normal range [0.25, 15.5] covers [0.125, 7.75] sigma: no clipping
# in practice (max|x| ~ 5.6), subnormal floor negligible.
SCALE = 2.0

MODE = "fp8e3"

_program_cache = {}


def _build_program_fp8(mode: str) -> bass.Bass:
    f32 = mybir.dt.float32
    f16 = mybir.dt.float16
    f8 = mybir.dt.float8e3
    nc = bacc.Bacc(None, name="dense_condenser")

    # xb[j, p, kt, b] = xq[j*CHUNK8 + b, kt*128 + p]: per (chunk, partition)
    # the (kt, b) payload is one contiguous 32 KiB fp8 run.
    xb = nc.dram_tensor("xb", (NCHUNK8, 128, NK, CHUNK8), f8, kind="ExternalInput")
    eb = nc.dram_tensor("eb", (128, NK, C), f16, kind="ExternalInput")
    # biasw[0, 32g+c] = bias[c] for g in 0..3, zeros elsewhere: the
    # stationary operand of the bank-opening broadcast matmul.
    biasw = nc.dram_tensor("biasw", (1, 128), f16, kind="ExternalInput")
    ones = nc.dram_tensor("ones", (1, NSLICE), f16, kind="ExternalInput")
    # full-width output staging: partition 32g+c, chunk ch, col b holds
    # out[ch*CHUNK8 + g*NSLICE + b, c]; partitions outside the 4 live
    # 8-row ranges carry bias junk the host discards. Storing all 128
    # partitions keeps it to ONE ~600ns HWDGE trigger per chunk instead
    # of 4 (the extra bytes are trivial: 64 KiB/chunk at 358 GB/s).
    # 104 partitions = through the last live row (group 3 at 96..104):
    # trimming the dead tail partitions cuts store descriptors ~20%.
    outF = nc.dram_tensor("outF", (104, NCHUNK8, NSLICE), f16, kind="ExternalOutput")

    with tile.TileContext(nc) as tc:
        with (
            tc.tile_pool(name="consts", bufs=1) as consts,
            tc.tile_pool(name="xp", bufs=5) as xp,
            tc.tile_pool(name="op", bufs=2) as op,
            tc.tile_pool(name="pp", bufs=2, space=bass.MemorySpace.PSUM) as pp,
            tc.tile_pool(name="pw", bufs=1, space=bass.MemorySpace.PSUM) as pw,
        ):
            e_tile = consts.tile([128, NK, C], f16)
            biasw_tile = consts.tile([1, 128], f16)
            ones_tile = consts.tile([1, NSLICE], f16)

            # x loads stream on the Sync HWDGE ring; consts ride the Scalar
            # ring so chunk 0's load is the very first thing the Sync ring
            # processes.
            x_tiles = []
            for j in range(NCHUNK8):
                x_tile = xp.tile([128, NK, CHUNK8], f8)
                x_tiles.append(x_tile)
                if j == 0:
                    nc.sync.dma_start(out=x_tile[:], in_=xb[j])
                    nc.scalar.dma_start(out=e_tile[:], in_=eb[:])
                    nc.scalar.dma_start(out=biasw_tile[:], in_=biasw[:])
                    nc.scalar.dma_start(out=ones_tile[:], in_=ones[:])
                elif j < NCHUNK8 - 1:
                    # whole-chunk loads: one 32 KiB contiguous run per
                    # partition is the DMA sweet spot (16 KiB and 64 KiB
                    # runs both measured slower).
                    nc.sync.dma_start(out=x_tile[:], in_=xb[j])
                else:
                    # final chunk in 16-ktile halves (16 KiB/partition runs;
                    # 8-ktile gates measured slower: their 8 KiB runs hit the
                    # degenerate single-DMA-engine path).
                    for lo, hi in ((0, 16), (16, 32)):
                        nc.sync.dma_start(
                            out=x_tile[:, lo:hi], in_=xb[j, :, lo:hi]
                        )

            warm_tile = pw.tile([128, NSLICE], f32)

            def filler_rounds(n, x_tile):
                # Redundant matmuls into a scratch PSUM bank. No consumers,
                # no waits: the PE runs them during what would otherwise be
                # DMA-bound idle, keeping the HAM activity window busy so
                # gated bursts run at 2.4 GHz instead of re-throttled 1.2.
                for _ in range(n):
                    nc.tensor.matmul(
                        warm_tile[:C, :],
                        e_tile[:, 0, :],
                        x_tile[:, 0, ts(0, NSLICE)],
                        start=True,
                        stop=True,
                        skip_group_check=True,
                        tile_position=(0, 0),
                    )

            for ch in range(NCHUNK8):
                x_tile = x_tiles[ch]
                psum_tile = pp.tile([128, NSLICE], f32)
                # Bank-wide opener: out[32g+c, b] = bias[c], has_written set
                # for every element of the bank so the 4 interleaved col
                # groups below can all accumulate with start=False.
                nc.tensor.matmul(
                    psum_tile[:],
                    biasw_tile[:],
                    ones_tile[:],
                    start=True,
                    stop=False,
                    skip_group_check=True,
                )
                # 4 col groups run concurrently (distinct 32-col array
                # strips + own XBUS streams): group g contracts k-tile kt
                # for batch slice g. kt-outer / g-inner issue order keeps
                # consecutive PE instructions on distinct groups.
                for kt in range(NK):
                    last = kt == NK - 1
                    for g in range(NGRP):
                        nc.tensor.matmul(
                            psum_tile[32 * g : 32 * g + C, :],
                            e_tile[:, kt, :],
                            x_tile[:, kt, ts(g, NSLICE)],
                            start=False,
                            stop=last,
                            skip_group_check=True,
                            tile_position=(0, 32 * g),
                        )
                    if ch == NCHUNK8 - 1 and kt == 15:
                        # bridge the gate-A -> gate-B idle (warm-state HAM
                        # re-throttles after ~1.7 us of PE idle)
                        filler_rounds(MID_FILL, x_tile)

                # One full-width DVE evacuation (psum partitions 8..31 etc.
                # hold bias junk; the host discards them). Downcast to fp16
                # (rel err ~5e-4, negligible vs the fp8 x quantization)
                # halves the store bytes. Evac on ScalarE measured slower
                # (its sequencer stall blocks the store triggers).
                out_tile = op.tile([128, NSLICE], f16, tag="out")
                nc.vector.tensor_scalar_add(out_tile[:], psum_tile[:], 0.0)
                # mid-run stores hide under the load stream on the Scalar
                # ring; the final one splits across both (by then idle)
                # rings so trigger + completion processing parallelize.
                if ch == NCHUNK8 - 1:
                    nc.sync.dma_start(out=outF[:64, ch, :], in_=out_tile[:64, :])
                    nc.scalar.dma_start(
                        out=outF[64:, ch, :], in_=out_tile[64:104, :]
                    )
                else:
                    nc.scalar.dma_start(out=outF[:, ch, :], in_=out_tile[:104, :])

    nc.compile()
    return nc


def _build_program_fp16(mode: str) -> bass.Bass:
    """Legacy fp16 program (see git history for rationale); kept as fallback."""
    f32 = mybir.dt.float32
    mmdt = mybir.dt.float16
    nc = bacc.Bacc(None, name="dense_condenser")

    xb = nc.dram_tensor("xb", (NCHUNK16, 128, NK, CHUNK16), mmdt, kind="ExternalInput")
    eb = nc.dram_tensor("eb", (128, NK, C), mmdt, kind="ExternalInput")
    bias = nc.dram_tensor("bias", (C, 1), f32, kind="ExternalInput")
    outT = nc.dram_tensor("outT", (C, B_CORE), f32, kind="ExternalOutput")

    with tile.TileContext(nc) as tc:
        with (
            tc.tile_pool(name="consts", bufs=1) as consts,
            tc.tile_pool(name="xp", bufs=5) as xp,
            tc.tile_pool(name="op", bufs=2) as op,
            tc.tile_pool(name="pp", bufs=2, space=bass.MemorySpace.PSUM) as pp,
        ):
            e_tile = consts.tile([128, NK, C], mmdt)
            bias_tile = consts.tile([C, 1], f32)

            x_tiles = []
            for j in range(NCHUNK16):
                x_tile = xp.tile([128, NK, CHUNK16], mmdt)
                x_tiles.append(x_tile)
                if j == 0:
                    nc.sync.dma_start(out=x_tile[:], in_=xb[j])
                    nc.scalar.dma_start(out=bias_tile[:], in_=bias[:])
                    nc.scalar.dma_start(out=e_tile[:], in_=eb[:])
                elif j < NCHUNK16 - 2:
                    nc.sync.dma_start(out=x_tile[:], in_=xb[j])
                else:
                    nc.sync.dma_start(out=x_tile[:, : NK // 2], in_=xb[j, :, : NK // 2])
                    nc.sync.dma_start(out=x_tile[:, NK // 2 :], in_=xb[j, :, NK // 2 :])

            GROUP = 4
            out_tile = None
            for j in range(NCHUNK16):
                x_tile = x_tiles[j]
                psum_tile = pp.tile([C, CHUNK16], f32)
                for kt in range(NK):
                    nc.tensor.matmul(
                        psum_tile[:],
                        e_tile[:, kt, :],
                        x_tile[:, kt, :],
                        start=(kt == 0),
                        stop=(kt == NK - 1),
                    )

                if j % GROUP == 0:
                    out_tile = op.tile([C, GROUP * CHUNK16], f32, tag="out")
                nc.vector.tensor_scalar_add(
                    out_tile[:, ts(j % GROUP, CHUNK16)], psum_tile[:], bias_tile[:]
                )
                if j % GROUP == GROUP - 1:
                    nc.scalar.dma_start(
                        out=outT[:, ts(j // GROUP, GROUP * CHUNK16)], in_=out_tile[:]
                    )

    nc.compile()
    return nc


def _fold_E(node_0, node_1, node_2) -> np.ndarray:
    # E[(i,j,k,l), c3] = sum_{c1,c2} node_0[l,k,c1] node_1[c1,j,c2] node_2[c2,i,c3]
    E = np.einsum(
        "lkc,cjd,die->ijkle",
        node_0.astype(np.float64),
        node_1.astype(np.float64),
        node_2.astype(np.float64),
    )
    return E.reshape(K, C).astype(np.float32)


def kernel(x, node_0, node_1, node_2, bias, _trace=False, _trace_cores=None):
    x = np.asarray(x, dtype=np.float32)
    E = _fold_E(np.asarray(node_0), np.asarray(node_1), np.asarray(node_2))
    bias_np = np.asarray(bias, dtype=np.float32)

    if MODE not in _program_cache:
        _program_cache[MODE] = (
            _build_program_fp8(MODE) if MODE == "fp8e3" else _build_program_fp16(MODE)
        )
    nc = _program_cache[MODE]

    in_maps = []
    if MODE == "fp8e3":
        # blocked E with the x-scale folded out: eb[p, kt, c] = E[kt*128+p, c]/SCALE
        ebq = np.ascontiguousarray(
            (E / SCALE).reshape(NK, 128, C).transpose(1, 0, 2)
        ).astype(np.float16)
        biasw = np.zeros((1, 128), dtype=np.float16)
        for g in range(NGRP):
            biasw[0, 32 * g : 32 * g + C] = bias_np.astype(np.float16)
        ones = np.ones((1, NSLICE), dtype=np.float16)

        xq = np.clip(x * SCALE, -15.5, 15.5).astype(ml_dtypes.float8_e3m4)
        for m in range(N_CORES):
            x_m = xq[m * B_CORE : (m + 1) * B_CORE, :]
            # xb[j, p, kt, b] = x_m[j*CHUNK8 + b, kt*128 + p]
            xb_m = np.ascontiguousarray(
                x_m.reshape(NCHUNK8, CHUNK8, NK, 128).transpose(0, 3, 2, 1)
            )
            in_maps.append({"xb": xb_m, "eb": ebq, "biasw": biasw, "ones": ones})
    else:
        eb = np.ascontiguousarray(E.reshape(NK, 128, C).transpose(1, 0, 2)).astype(
            np.float16
        )
        bias_col = bias_np.reshape(C, 1)
        for m in range(N_CORES):
            x_m = x[m * B_CORE : (m + 1) * B_CORE, :]
            xb_m = x_m.reshape(NCHUNK16, CHUNK16, NK, 128).transpose(0, 3, 2, 1)
            xb_m = xb_m.astype(np.float16)
            in_maps.append({"xb": xb_m, "eb": eb, "bias": bias_col})

    res = run_bass_kernel_spmd(
        nc,
        in_maps,
        core_ids=list(range(N_CORES)),
        trace=_trace,
        trace_cores=_trace_cores,
    )
    results = res.results

    out = np.empty((BATCH, C), dtype=np.float32)
    for m in range(N_CORES):
        if MODE == "fp8e3":
            # outF[32g+c, ch, b] -> out[ch*CHUNK8 + g*NSLICE + b, c]
            of = results[m]["outF"]  # (104, NCHUNK8, NSLICE)
            arr = np.stack([of[32 * g : 32 * g + C] for g in range(NGRP)])
            out[m * B_CORE : (m + 1) * B_CORE, :] = (
                arr.transpose(2, 0, 3, 1).reshape(B_CORE, C).astype(np.float32)
            )
        else:
            out[m * B_CORE : (m + 1) * B_CORE, :] = results[m]["outT"].T.astype(
                np.float32
            )

    if _trace:
        return out, res
    return out


# revision 47
# speedup vs baseline: 1.1644x; 1.0402x over previous
"""Trainium2 Bass kernel for nn_DenseCondenser (TT contraction, 65536x4096 -> 65536x8).

The three (8,8,8) TT cores compose into a single effective matrix E (4096, 8)
(the whole map is linear in x), folded on host in float64. The device kernel
is then a memory-bound skinny matmul out = x @ E + bias, data-parallel over
the batch across 8 NeuronCores.

MODE "fp8e3" (default): x is cast on host to fp8 e3m4 (Trainium FP8_EXP3,
4 mantissa bits) with a power-of-2 scale folded into E; E stays fp16
(TensorE allows mixed input dtypes; both upcast to ~fp22 internally).
This halves HBM traffic vs fp16 (L2 rel err 1.332e-2 vs the 2e-2 gate;
fp16 was 2.9e-4). At 1 B/elem the PE streaming time (1 col/cycle, only
8 of 128 array columns used) would exceed the DMA time, so the matmuls
are packed 4x with PE column tiling: col group g (tile_position=(0,32g))
processes batch slice g of the chunk, writing psum partitions 32g..32g+8
(no cross-group combine needed - the groups are just different batch
rows). A single full-width (M=128) bias-broadcast matmul opens each PSUM
bank (start=True clears has_written for the WHOLE bank, so it must happen
exactly once per bank, before all 4 groups' accumulating matmuls). DMA
cannot read PSUM, so one full-width DVE copy (fp32 psum -> fp16 sbuf)
evacuates each chunk, then ONE 128-partition store per chunk writes the
staging tensor outF (junk partitions included - trivial bytes, and 1
HWDGE trigger at ~0.6 us each beats 4); the host picks the live rows.

Device-side layout: x is staged per-core host-blocked as
xb (8 chunks, 128 partitions, 32 ktiles, 1024 batch) so the contraction
dim lands on SBUF partitions and every (chunk, partition) DMA payload is
one contiguous 32 KiB fp8 run (16 KiB and 64 KiB runs both measured
slower; 8 KiB runs hit a degenerate single-DMA-engine path). All chunk
loads are hoisted up front on the Sync HWDGE ring (bufs=5; bufs=6
regressed); mid-run stores ride the Scalar ring, the final store splits
across both then-idle rings so trigger + completion parallelize. The
last chunk loads in 16-ktile halves so only 16 rounds remain after the
final byte, and MID_FILL scratch matmuls keep the PE's HAM clock gate
warm across the inter-gate idle.

Measured (8-way SPMD, profiled core): 107.2 us best / ~14.8 us overhead
above the DMA floor; per-core sustained load rate swings 92-109 us for
the same 32 MiB with chip load/thermals, so absolute exec varies
run-to-run. History: fp32 363 -> fp16 210-223 -> fp8e3 107-118.
NOTE: per-chunk warm-keeper fillers, 8-ktile gates, scalar-engine evac,
and store-per-group variants each measured SLOWER - see git history.
"""

import numpy as np
import ml_dtypes

import concourse.bass as bass
import concourse.mybir as mybir
import concourse.tile as tile
from concourse import bacc
from concourse.bass import ts
from concourse.bass_utils import run_bass_kernel_spmd

# Problem shapes (hardcoded per harness contract)
BATCH = 65536
K = 4096  # input features = 8**4
C = 8  # output features
N_CORES = 8
B_CORE = BATCH // N_CORES  # 8192
NK = K // 128  # 32 k-tiles

# fp8e3 mode geometry: 1024-batch chunks, 4 PE col groups x 256-batch slices
CHUNK8 = 1024
NCHUNK8 = B_CORE // CHUNK8  # 8
NGRP = 4
NSLICE = CHUNK8 // NGRP  # 256
# filler matmuls bridging the last chunk's gate-A -> gate-B PE idle: the
# HAM clock gate re-throttles the PE to 1.2 GHz after ~1.7 us of warm-state
# idle, and the final 16 load-gated rounds are on the critical path. 12
# rounds (~1.4 us) keep the idle under the window. Fillers anywhere else
# (per-chunk) measurably SLOW THE DMA STREAM (~+5 us) - do not add them.
MID_FILL = 12

# fp16 mode geometry (legacy fallback)
CHUNK16 = 512
NCHUNK16 = B_CORE // CHUNK16  # 16

# x quantization scale for fp8e3 (power of 2, folded into E). At s=2 the
# e3m4 normal range [0.25, 15.5] covers [0.125, 7.75] sigma: no clipping
# in practice (max|x| ~ 5.6), subnormal floor negligible.
SCALE = 2.0

MODE = "fp8e3"

_program_cache = {}


def _build_program_fp8(mode: str) -> bass.Bass:
    f32 = mybir.dt.float32
    f16 = mybir.dt.float16
    f8 = mybir.dt.float8e3
    nc = bacc.Bacc(None, name="dense_condenser")

    # xb[j, p, kt, b] = xq[j*CHUNK8 + b, kt*128 + p]: per (chunk, partition)
    # the (kt, b) payload is one contiguous 32 KiB fp8 run.
    xb = nc.dram_tensor("xb", (NCHUNK8, 128, NK, CHUNK8), f8, kind="ExternalInput")
    eb = nc.dram_tensor("eb", (128, NK, C), f16, kind="ExternalInput")
    # biasw[0, 32g+c] = bias[c] for g in 0..3, zeros elsewhere: the
    # stationary operand of the bank-opening broadcast matmul.
    biasw = nc.dram_tensor("biasw", (1, 128), f16, kind="ExternalInput")
    ones = nc.dram_tensor("ones", (1, NSLICE), f16, kind="ExternalInput")
    # full-width output staging: partition 32g+c, chunk ch, col b holds
    # out[ch*CHUNK8 + g*NSLICE + b, c]; partitions outside the 4 live
    # 8-row ranges carry bias junk the host discards. Storing all 128
    # partitions keeps it to ONE ~600ns HWDGE trigger per chunk instead
    # of 4 (the extra bytes are trivial: 64 KiB/chunk at 358 GB/s).
    # 104 partitions = through the last live row (group 3 at 96..104):
    # trimming the dead tail partitions cuts store descriptors ~20%.
    outF = nc.dram_tensor("outF", (104, NCHUNK8, NSLICE), f16, kind="ExternalOutput")

    with tile.TileContext(nc) as tc:
        with (
            tc.tile_pool(name="consts", bufs=1) as consts,
            tc.tile_pool(name="xp", bufs=5) as xp,
            tc.tile_pool(name="op", bufs=2) as op,
            tc.tile_pool(name="pp", bufs=2, space=bass.MemorySpace.PSUM) as pp,
            tc.tile_pool(name="pw", bufs=1, space=bass.MemorySpace.PSUM) as pw,
        ):
            e_tile = consts.tile([128, NK, C], f16)
            biasw_tile = consts.tile([1, 128], f16)
            ones_tile = consts.tile([1, NSLICE], f16)

            # x loads stream on the Sync HWDGE ring; consts ride the Scalar
            # ring so chunk 0's load is the very first thing the Sync ring
            # processes.
            x_tiles = []
            for j in range(NCHUNK8):
                x_tile = xp.tile([128, NK, CHUNK8], f8)
                x_tiles.append(x_tile)
                if j == 0:
                    nc.sync.dma_start(out=x_tile[:], in_=xb[j])
                    nc.scalar.dma_start(out=e_tile[:], in_=eb[:])
                    nc.scalar.dma_start(out=biasw_tile[:], in_=biasw[:])
                    nc.scalar.dma_start(out=ones_tile[:], in_=ones[:])
                elif j < NCHUNK8 - 1:
                    # whole-chunk loads: one 32 KiB contiguous run per
                    # partition is the DMA sweet spot (16 KiB and 64 KiB
                    # runs both measured slower).
                    nc.sync.dma_start(out=x_tile[:], in_=xb[j])
                else:
                    # final chunk in 16-ktile halves (16 KiB/partition runs;
                    # 8-ktile gates measured slower: their 8 KiB runs hit the
                    # degenerate single-DMA-engine path).
                    for lo, hi in ((0, 16), (16, 32)):
                        nc.sync.dma_start(
                            out=x_tile[:, lo:hi], in_=xb[j, :, lo:hi]
                        )

            warm_tile = pw.tile([128, NSLICE], f32)

            def filler_rounds(n, x_tile):
                # Redundant matmuls into a scratch PSUM bank. No consumers,
                # no waits: the PE runs them during what would otherwise be
                # DMA-bound idle, keeping the HAM activity window busy so
                # gated bursts run at 2.4 GHz instead of re-throttled 1.2.
                for _ in range(n):
                    nc.tensor.matmul(
                        warm_tile[:C, :],
                        e_tile[:, 0, :],
                        x_tile[:, 0, ts(0, NSLICE)],
                        start=True,
                        stop=True,
                        skip_group_check=True,
                        tile_position=(0, 0),
                    )

            for ch in range(NCHUNK8):
                x_tile = x_tiles[ch]
                psum_tile = pp.tile([128, NSLICE], f32)
                # Bank-wide opener: out[32g+c, b] = bias[c], has_written set
                # for every element of the bank so the 4 interleaved col
                # groups below can all accumulate with start=False.
                nc.tensor.matmul(
                    psum_tile[:],
                    biasw_tile[:],
                    ones_tile[:],
                    start=True,
                    stop=False,
                    skip_group_check=True,
                )
                # 4 col groups run concurrently (distinct 32-col array
                # strips + own XBUS streams): group g contracts k-tile kt
                # for batch slice g. kt-outer / g-inner issue order keeps
                # consecutive PE instructions on distinct groups.
                for kt in range(NK):
                    last = kt == NK - 1
                    for g in range(NGRP):
                        nc.tensor.matmul(
                            psum_tile[32 * g : 32 * g + C, :],
                            e_tile[:, kt, :],
                            x_tile[:, kt, ts(g, NSLICE)],
                            start=False,
                            stop=last,
                            skip_group_check=True,
                            tile_position=(0, 32 * g),
                        )
                    if ch == NCHUNK8 - 1 and kt == 15:
                        # bridge the gate-A -> gate-B idle (warm-state HAM
                        # re-throttles after ~1.7 us of PE idle)
                        filler_rounds(MID_FILL, x_tile)

                # One full-width DVE evacuation (psum partitions 8..31 etc.
                # hold bias junk; the host discards them). Downcast to fp16
                # (rel err ~5e-4, negligible vs the fp8 x quantization)
                # halves the store bytes. Evac on ScalarE measured slower
                # (its sequencer stall blocks the store triggers).
                out_tile = op.tile([128, NSLICE], f16, tag="out")
                nc.vector.tensor_scalar_add(out_tile[:], psum_tile[:], 0.0)
                # mid-run stores hide under the load stream on the Scalar
                # ring; the final one splits across both (by then idle)
                # rings so trigger + completion processing parallelize.
                if ch == NCHUNK8 - 1:
                    nc.sync.dma_start(out=outF[:64, ch, :], in_=out_tile[:64, :])
                    nc.scalar.dma_start(
                        out=outF[64:, ch, :], in_=out_tile[64:104, :]
                    )
                else:
                    nc.scalar.dma_start(out=outF[:, ch, :], in_=out_tile[:104, :])

    nc.compile()
    return nc


def _build_program_fp16(mode: str) -> bass.Bass:
    """Legacy fp16 program (see git history for rationale); kept as fallback."""
    f32 = mybir.dt.float32
    mmdt = mybir.dt.float16
    nc = bacc.Bacc(None, name="dense_condenser")

    xb = nc.dram_tensor("xb", (NCHUNK16, 128, NK, CHUNK16), mmdt, kind="ExternalInput")
    eb = nc.dram_tensor("eb", (128, NK, C), mmdt, kind="ExternalInput")
    bias = nc.dram_tensor("bias", (C, 1), f32, kind="ExternalInput")
    outT = nc.dram_tensor("outT", (C, B_CORE), f32, kind="ExternalOutput")

    with tile.TileContext(nc) as tc:
        with (
            tc.tile_pool(name="consts", bufs=1) as consts,
            tc.tile_pool(name="xp", bufs=5) as xp,
            tc.tile_pool(name="op", bufs=2) as op,
            tc.tile_pool(name="pp", bufs=2, space=bass.MemorySpace.PSUM) as pp,
        ):
            e_tile = consts.tile([128, NK, C], mmdt)
            bias_tile = consts.tile([C, 1], f32)

            x_tiles = []
            for j in range(NCHUNK16):
                x_tile = xp.tile([128, NK, CHUNK16], mmdt)
                x_tiles.append(x_tile)
                if j == 0:
                    nc.sync.dma_start(out=x_tile[:], in_=xb[j])
                    nc.scalar.dma_start(out=bias_tile[:], in_=bias[:])
                    nc.scalar.dma_start(out=e_tile[:], in_=eb[:])
                elif j < NCHUNK16 - 2:
                    nc.sync.dma_start(out=x_tile[:], in_=xb[j])
                else:
                    nc.sync.dma_start(out=x_tile[:, : NK // 2], in_=xb[j, :, : NK // 2])
                    nc.sync.dma_start(out=x_tile[:, NK // 2 :], in_=xb[j, :, NK // 2 :])

            GROUP = 4
            out_tile = None
            for j in range(NCHUNK16):
                x_tile = x_tiles[j]
                psum_tile = pp.tile([C, CHUNK16], f32)
                for kt in range(NK):
                    nc.tensor.matmul(
                        psum_tile[:],
                        e_tile[:, kt, :],
                        x_tile[:, kt, :],
                        start=(kt == 0),
                        stop=(kt == NK - 1),
                    )

                if j % GROUP == 0:
                    out_tile = op.tile([C, GROUP * CHUNK16], f32, tag="out")
                nc.vector.tensor_scalar_add(
                    out_tile[:, ts(j % GROUP, CHUNK16)], psum_tile[:], bias_tile[:]
                )
                if j % GROUP == GROUP - 1:
                    nc.scalar.dma_start(
                        out=outT[:, ts(j // GROUP, GROUP * CHUNK16)], in_=out_tile[:]
                    )

    nc.compile()
    return nc


def _fold_E(node_0, node_1, node_2) -> np.ndarray:
    # E[(i,j,k,l), c3] = sum_{c1,c2} node_0[l,k,c1] node_1[c1,j,c2] node_2[c2,i,c3]
    E = np.einsum(
        "lkc,cjd,die->ijkle",
        node_0.astype(np.float64),
        node_1.astype(np.float64),
        node_2.astype(np.float64),
    )
    return E.reshape(K, C).astype(np.float32)


def kernel(x, node_0, node_1, node_2, bias, _trace=False, _trace_cores=None):
    x = np.asarray(x, dtype=np.float32)
    E = _fold_E(np.asarray(node_0), np.asarray(node_1), np.asarray(node_2))
    bias_np = np.asarray(bias, dtype=np.float32)

    if MODE not in _program_cache:
        _program_cache[MODE] = (
            _build_program_fp8(MODE) if MODE == "fp8e3" else _build_program_fp16(MODE)
        )
    nc = _program_cache[MODE]

    in_maps = []
    if MODE == "fp8e3":
        # blocked E with the x-scale folded out: eb[p, kt, c] = E[kt*128+p, c]/SCALE
        ebq = np.ascontiguousarray(
            (E / SCALE).reshape(NK, 128, C).transpose(1, 0, 2)
        ).astype(np.float16)
        biasw = np.zeros((1, 128), dtype=np.float16)
        for g in range(NGRP):
            biasw[0, 32 * g : 32 * g + C] = bias_np.astype(np.float16)
        ones = np.ones((1, NSLICE), dtype=np.float16)

        xq = np.clip(x * SCALE, -15.5, 15.5).astype(ml_dtypes.float8_e3m4)
        for m in range(N_CORES):
            x_m = xq[m * B_CORE : (m + 1) * B_CORE, :]
            # xb[j, p, kt, b] = x_m[j*CHUNK8 + b, kt*128 + p]
            xb_m = np.ascontiguousarray(
                x_m.reshape(NCHUNK8, CHUNK8, NK, 128).transpose(0, 3, 2, 1)
            )
            in_maps.append({"xb": xb_m, "eb": ebq, "biasw": biasw, "ones": ones})
    else:
        eb = np.ascontiguousarray(E.reshape(NK, 128, C).transpose(1, 0, 2)).astype(
            np.float16
        )
        bias_col = bias_np.reshape(C, 1)
        for m in range(N_CORES):
            x_m = x[m * B_CORE : (m + 1) * B_CORE, :]
            xb_m = x_m.reshape(NCHUNK16, CHUNK16, NK, 128).transpose(0, 3, 2, 1)
            xb_m = xb_m.astype(np.float16)
            in_maps.append({"xb": xb_m, "eb": eb, "bias": bias_col})

    res = run_bass_kernel_spmd(
        nc,
        in_maps,
        core_ids=list(range(N_CORES)),
        trace=_trace,
        trace_cores=_trace_cores,
    )
    results = res.results

    out = np.empty((BATCH, C), dtype=np.float32)
    for m in range(N_CORES):
        if MODE == "fp8e3":
            # outF[32g+c, ch, b] -> out[ch*CHUNK8 + g*NSLICE + b, c]
            of = results[m]["outF"]  # (104, NCHUNK8, NSLICE)
            arr = np.stack([of[32 * g : 32 * g + C] for g in range(NGRP)])
            out[m * B_CORE : (m + 1) * B_CORE, :] = (
                arr.transpose(2, 0, 3, 1).reshape(B_CORE, C).astype(np.float32)
            )
        else:
            out[m * B_CORE : (m + 1) * B_CORE, :] = results[m]["outT"].T.astype(
                np.float32
            )

    if _trace:
        return out, res
    return out
